# revision 24
# baseline (speedup 1.0000x reference)
"""CapsuleLayer dynamic-routing kernel for Trainium2 (Bass/Tile), SPMD over 8 cores.

Problem (per full input):
  x:  [256, 1152, 8]   route_weights: [10, 1152, 8, 16]
  priors[c,b,n,o] = sum_i x[b,n,i] * W[c,n,i,o]
  3 routing iterations; logits along o are constant =>
  probs are per-(c,b,n) scalars.  out: [10, 256, 1, 1, 16]

Math (per core, b = 32 local batch):
  logits_t[c,b,n] = priors[c,b,n,:] . a_t[c,b,:], a_t = sum of previous
  squashed outputs.  l = sum_i x * V,  V = sum_o W a  (PE matmul, k=(c,o)).
  s_raw[c,b,o] = sum_{n,i} e^l x W  (PE matmul over k=(i,n); 1/Z folded
  into the squash scale).  exp shift keeps e^l in fp16 range.

v3 engine plan (v1 was Vector-bound at 66% with 1x-rate strided ops):
  - x stored once; every consumer reads it through stride-0 broadcast APs
    (verified supported on both DVE and PE operands) -> no 5x replication
    in HBM or SBUF.
  - V evacuated PSUM->fp16 SBUF on the Scalar engine; Vector runs one
    contiguous fp16 multiply at 2x per group-pass and a 3-level tree-add
    for sum_i (tensor_reduce is 1x-only and was strided).
  - 3 input DMAs (fp16 blob = consts|wk|x, wt, tiny fp32 consts); wt
    streams during iter-0 compute.
  - squash per group so the next group's V-matmuls overlap it on PE.
  - PSUM: vb 3 banks x2, spsum 1 bank, one rotating 1-bank "sm" tile for
    small matmul outputs (8 banks total); PSUM reads all at partition 0.
  - capsules in two groups of 5, packed (c,o) x (c',b); off-diag blocks
    masked in the a accumulator (makes the packed V-matmul exact).
"""

import os
from contextlib import ExitStack

import numpy as np

B, N, CI, CO, NCAPS = 256, 1152, 8, 16, 10
NCORES = 8
BL = B // NCORES          # 32 batch per core
NB = N // 128             # 9 n-blocks
NCH = CI * NB             # 72 k-chunks, j = i*9 + nb
CG = 5                    # capsules per group
KO = CG * CO              # 80 partition rows in (c,o) layout
FB = CG * BL              # 160 free cols in (c',b) layout
NUM_ITERATIONS = 3

# fp16 const-blob column map
CB_COLS = 448
MASK0 = 0                 # mask2 [80, 2*160] at cols 0:320
ONESNL = 320              # ones_nl fp16 [128, 1]
# fp32 consts column map
EXPB1, EXPB2 = 0, 1       # exp shifts (0.0 / -9.5)
BD0 = 2                   # bdones [80, 5]  cols 2:7
SEL0 = 7                  # selc  [5, 80]   cols 7:87
ONE1C = 87                # ones_1c [1, 5]  cols 87:92
C32_COLS = 96

BLOB_COLS = CB_COLS + NCH * NCAPS * CO + NCH * BL   # 448+11520+2304

_compiled = None


def _build():
    import concourse.bacc as bacc
    import concourse.mybir as mybir
    import concourse.tile as tile

    fp32 = mybir.dt.float32
    fp16 = mybir.dt.float16
    bf16 = mybir.dt.bfloat16
    Alu = mybir.AluOpType
    Act = mybir.ActivationFunctionType

    nc = bacc.Bacc("TRN2", target_bir_lowering=False, debug=False)
    blobA_d = nc.dram_tensor("blobA", [128, CB_COLS + NCH * BL], bf16,
                             kind="ExternalInput")
    wkb_d = nc.dram_tensor("wkb", [128, NCH * NCAPS * CO], bf16,
                           kind="ExternalInput")
    xf_d = nc.dram_tensor("xf", [128, NCH * BL], fp16, kind="ExternalInput")
    wt_d = nc.dram_tensor("wt", [KO, 2 * NCH * 128], fp16, kind="ExternalInput")
    c32_d = nc.dram_tensor("c32", [128, C32_COLS], fp32, kind="ExternalInput")
    o_d = nc.dram_tensor("out", [KO, 2, FB], fp32, kind="ExternalOutput")

    with tile.TileContext(nc) as tc, ExitStack() as ctx:
        P = ctx.enter_context(tc.tile_pool(name="persist", bufs=1))
        vbp = ctx.enter_context(tc.tile_pool(name="vbp", bufs=3))
        xvp = ctx.enter_context(tc.tile_pool(name="xvp", bufs=1))
        xpp = ctx.enter_context(tc.tile_pool(name="xpp", bufs=2))
        smallp = ctx.enter_context(tc.tile_pool(name="small", bufs=1))
        psv = ctx.enter_context(tc.tile_pool(name="psv", bufs=2, space="PSUM"))
        pss = ctx.enter_context(tc.tile_pool(name="pss", bufs=1, space="PSUM"))

        # ---- persistent SBUF ----
        blobA = P.tile([128, CB_COLS + NCH * BL], bf16)
        wkt = P.tile([128, NCH, NCAPS * CO], bf16)
        xf = P.tile([128, NCH, BL], fp16)
        c32 = P.tile([128, C32_COLS], fp32)
        wt = P.tile([KO, 2, NCH, 128], fp16)
        xc = P.tile([128, NCH, CG, BL], fp16)  # x replicated over c'
        a_all = P.tile([KO, 2, FB], fp32)
        a_bf = P.tile([KO, 2, FB], fp16)
        elt = P.tile([128, 2, NB, FB], bf16)       # e^l per group

        wk = wkt[:, :, :]                                  # [128, 72, 160]
        xh = blobA[:, CB_COLS:] \
            .rearrange("p (j b) -> p j b", j=NCH)          # [128, 72, 32]
        mask2 = blobA[0:KO, MASK0:MASK0 + 2 * FB]          # [80, 320] bf16
        ones16 = blobA[:, ONESNL:ONESNL + 1]               # [128, 1] bf16
        expb1 = c32[:, EXPB1:EXPB1 + 1]
        expb2 = c32[:, EXPB2:EXPB2 + 1]
        bdones = c32[0:KO, BD0:BD0 + CG]                   # [80, 5] fp32
        selc = c32[0:CG, SEL0:SEL0 + KO]                   # [5, 80] fp32
        ones1c = c32[0:1, ONE1C:ONE1C + CG]                # [1, 5] fp32

        def xbc(j0, j1):
            """bf16 x chunk [128, j1-j0, 5, 32] broadcast over capsules."""
            return xh[:, j0:j1, :].unsqueeze(2).broadcast_to(
                [128, j1 - j0, CG, BL])

        def xbf(j0, j1):
            """fp16 x chunk broadcast over capsules (V path)."""
            return xf[:, j0:j1, :].unsqueeze(2).broadcast_to(
                [128, j1 - j0, CG, BL])

        # ---- input DMAs (two HWDGE queues) ----
        nc.scalar.dma_start(c32[:], c32_d[:])
        nc.sync.dma_start(blobA[:], blobA_d[:])
        H = NCH // 2
        nc.sync.dma_start(wkt[:, 0:H, :], wkb_d[:, 0:H * NCAPS * CO]
                          .rearrange("p (j c) -> p j c", j=H))
        nc.sync.dma_start(wkt[:, H:, :], wkb_d[:, H * NCAPS * CO:]
                          .rearrange("p (j c) -> p j c", j=H))
        nc.scalar.dma_start(xf.rearrange("p j b -> p (j b)"), xf_d[:])
        nc.scalar.dma_start(wt[:, 0, :, :].rearrange("p j n -> p (j n)"),
                            wt_d[:, 0:NCH * 128])
        nc.scalar.dma_start(wt[:, 1, :, :].rearrange("p j n -> p (j n)"),
                            wt_d[:, NCH * 128:])

        for cr in range(CG):
            nc.vector.tensor_scalar_mul(xc[:, :, cr, :], xf[:, :, :], 1.0)

        rz_cur = [None, None]

        def sm_tile():
            return pss.tile([128, 512], fp32, tag="sm", name="sm")

        def squash(it, g, ssb, s2):
            """snp/frp via PE; writes a/a_bf (or output DMA at last iter)."""
            smt = sm_tile()
            snp = smt[0:CG, 0:FB]
            nc.tensor.matmul(snp, bdones, s2, start=True, stop=True)
            snt = smallp.tile([CG, FB], fp32, tag="snt")
            if it == 0:
                zc = 1.0 / N
                nc.vector.tensor_scalar_mul(snt[:], snp, zc * zc)
            else:
                rzc = rz_cur[g][:, :]
                t1 = smallp.tile([CG, FB], fp32, tag="snt_t1")
                nc.vector.tensor_tensor(t1[:], rzc, snp, Alu.mult)
                nc.vector.tensor_tensor(snt[:], t1[:], rzc, Alu.mult)
            sq = smallp.tile([CG, FB], fp32, tag="sq")
            nc.scalar.sqrt(sq[:], snt[:])
            den = smallp.tile([CG, FB], fp32, tag="den")
            nc.scalar.add(den[:], snt[:], 1.0)
            rden = smallp.tile([CG, FB], fp32, tag="rden")
            nc.vector.reciprocal(rden[:], den[:])
            g_t = smallp.tile([CG, FB], fp32, tag="g")
            nc.vector.tensor_tensor(g_t[:], sq[:], rden[:], Alu.mult)
            f = smallp.tile([CG, FB], fp32, tag="f")
            if it == 0:
                nc.vector.tensor_scalar_mul(f[:], g_t[:], 1.0 / N)
            else:
                nc.vector.tensor_tensor(f[:], g_t[:], rz_cur[g][:, :],
                                        Alu.mult)
            smf = sm_tile()
            frp = smf[0:KO, 0:FB]
            nc.tensor.matmul(frp, selc, f[:], start=True, stop=True)
            ov = smallp.tile([KO, FB], fp32, tag="ov")
            nc.vector.tensor_tensor(ov[:], ssb, frp, Alu.mult)
            aq = a_all[:, g, :]
            if it == 0:
                nc.vector.tensor_tensor(aq, ov[:],
                                        mask2[:, g * FB:(g + 1) * FB], Alu.mult)
                nc.scalar.copy(a_bf[:, g, :], aq)
            elif it < NUM_ITERATIONS - 1:
                t2 = smallp.tile([KO, FB], fp32, tag="aq_t")
                nc.vector.tensor_tensor(t2[:], aq, ov[:], Alu.add)
                nc.vector.tensor_tensor(aq, t2[:],
                                        mask2[:, g * FB:(g + 1) * FB], Alu.mult)
                nc.scalar.copy(a_bf[:, g, :], aq)
            else:
                nc.sync.dma_start(o_d[:, g, :], ov[:])

        # ---- PE warm-up: dummy matmuls on c32 while input DMAs stream;
        # keeps HAM at K=8/8 so iter-0 runs at 2.4 GHz ----
        smw = sm_tile()
        for r in range(48):
            nc.tensor.matmul(smw[0:80, 0:96], c32[:, 0:80], c32[:, 0:96],
                             start=True, stop=True)

        # ================= iteration 0 =================
        # s0[(c,o),(c',b)] = sum_j W^T x  (x broadcast over c'), both groups
        # into one PSUM bank; squash reads only the diagonal blocks.
        # one accumulation chain open per PSUM bank at a time (interleaved
        # chains corrupt each other), so run the groups sequentially
        sps0 = pss.tile([KO, 2 * FB], fp32, tag="spsum")
        it0_g1 = [None]
        for g in (0, 1):
            for j in range(NCH):
                xbj = xh[:, j, :].unsqueeze(1).broadcast_to([128, CG, BL])
                nc.tensor.matmul(sps0[:, g * FB:(g + 1) * FB],
                                 wk[:, j, g * KO:(g + 1) * KO], xbj,
                                 start=(j == 0), stop=(j == NCH - 1))
            ssb = smallp.tile([KO, FB], fp32, tag="ssb")
            nc.scalar.copy(ssb[:], sps0[:, g * FB:(g + 1) * FB])
            s2 = smallp.tile([KO, FB], fp32, tag="s2")
            nc.vector.tensor_tensor(s2[:], ssb[:], ssb[:], Alu.mult)
            if g == 0:
                # g0 squash hides under g1's matmul chain -> a_bf0 ready
                squash(0, g, ssb[:], s2[:])
            else:
                # defer g1's squash past l_pass(0, it=1) so V0 isn't
                # FIFO-blocked behind its snp/frp matmuls
                it0_g1[0] = (ssb, s2)

        # ================= iterations 1, 2 =================
        zsb_cur = [None]
        sps_cur = [None]

        def l_pass(g, it, filler=None):
            xv = xvp.tile([128, CI, NB * FB], fp16, tag="xv")
            for i in range(CI):
                vb = psv.tile([128, 3, 512], fp32, tag="vb")
                for nbg in range(3):
                    for k in range(3):
                        j = i * NB + nbg * 3 + k
                        nc.tensor.matmul(vb[:, nbg, k * FB:(k + 1) * FB],
                                         wt[:, g, j, :], a_bf[:, g, :],
                                         start=True, stop=True)
                vbf = vbp.tile([128, NB * FB], fp16, tag="vbf")
                nc.scalar.copy(
                    vbf.rearrange("p (a b) -> p a b", a=3),
                    vb[:, :, 0:3 * FB])
                # xv_i = x * V_i, contiguous fp16 at 2x
                nc.vector.tensor_tensor(
                    xv[:, i, :].rearrange("p (a f) -> p a f", a=NB),
                    xc[:, i * NB:(i + 1) * NB, :, :]
                    .rearrange("p j c b -> p j (c b)"),
                    vbf.rearrange("p (a f) -> p a f", a=NB),
                    Alu.mult)
                if filler is not None:
                    filler(i)   # independent DVE work fills the evac wait
            # tree-add over i: [8, 9*160]
            xvi = xv.rearrange("p i f -> p (i f)") \
                    .rearrange("p (i f) -> p i f", i=CI)
            t1 = xvp.tile([128, 4, NB * FB], fp16, tag="t1")
            nc.vector.tensor_tensor(t1[:], xvi[:, 0:4, :], xvi[:, 4:8, :],
                                    Alu.add)
            nc.vector.tensor_tensor(xvi[:, 0:2, :], t1[:, 0:2, :],
                                    t1[:, 2:4, :], Alu.add)
            nc.vector.tensor_tensor(xvi[:, 2, :], xvi[:, 0, :], xvi[:, 1, :],
                                    Alu.add)
            nc.scalar.activation(
                elt[:, g, :, :].rearrange("p a b -> p (a b)"), xvi[:, 2, :],
                Act.Exp, bias=(expb1 if it == 1 else expb2))

        def z_pass(g):
            # Z[g] = sum_n e^l, then rz[g] = 1/Z broadcast to CG rows;
            # per group so each squash chain can start early
            smz = sm_tile()
            zpg = smz[0:1, 0:FB]
            for nb in range(NB):
                nc.tensor.matmul(zpg, ones16, elt[:, g, nb, :],
                                 start=(nb == 0), stop=(nb == NB - 1))
            zsb = smallp.tile([1, FB], fp32, tag="zsb" + str(g), name="zsb")
            nc.scalar.copy(zsb[:], zpg)
            rzg = smallp.tile([1, FB], fp32, tag="rzg" + str(g), name="rzg")
            nc.vector.reciprocal(rzg[:], zsb[:])
            smr = sm_tile()
            rzcp = smr[0:CG, 0:FB]
            nc.tensor.matmul(rzcp, ones1c, rzg[:], start=True, stop=True)
            rzc = smallp.tile([CG, FB], fp32, tag="rzc" + str(g), name="rzc")
            nc.scalar.copy(rzc[:], rzcp)
            rz_cur[g] = rzc

        def xe_op(g, xp, i):
            # xe_i = x * e^l, contiguous (fp16 x bf16 runs at 2x)
            nc.vector.tensor_tensor(
                xp[:, i * NB:(i + 1) * NB, :],
                xc[:, i * NB:(i + 1) * NB, :, :]
                .rearrange("p j c b -> p j (c b)"),
                elt[:, g, :, :], Alu.mult)

        def xe_pass(g):
            xp = xpp.tile([128, NCH, FB], bf16, tag="xp")
            for i in range(CI):
                xe_op(g, xp, i)
            return xp

        def s_mm(g, xp):
            for j in range(NCH):
                nc.tensor.matmul(
                    sps_cur[0][:, g * FB:(g + 1) * FB],
                    wk[:, j, g * KO:(g + 1) * KO], xp[:, j, :],
                    start=(j == 0), stop=(j == NCH - 1))
            ssb = smallp.tile([KO, FB], fp32, tag="ssb" + str(g))
            nc.scalar.copy(ssb[:], sps_cur[0][:, g * FB:(g + 1) * FB])
            return ssb

        def s_squash(g, it, ssb):
            s2 = smallp.tile([KO, FB], fp32, tag="s2")
            nc.vector.tensor_tensor(s2[:], ssb[:], ssb[:], Alu.mult)
            squash(it, g, ssb[:], s2[:])

        for it in (1, 2):
            sps_cur[0] = pss.tile([KO, 2 * FB], fp32, tag="spsum",
                                  name="sps")
            # V0,V1 dense on PE; xe0 fills l1's evac-paced DVE gaps;
            # per-group rz lets squash0 run early so a_bf0 is ready
            # before the next iteration's V0 matmuls
            l_pass(0, it)
            if it == 1:
                ssbg1, s2g1 = it0_g1[0]
                squash(0, 1, ssbg1[:], s2g1[:])
            xp0 = xpp.tile([128, NCH, FB], bf16, tag="xp", name="xp0")
            l_pass(1, it, filler=lambda i: xe_op(0, xp0, i))
            z_pass(0)
            ssb0 = s_mm(0, xp0)
            s_squash(0, it, ssb0)
            xp1 = xe_pass(1)
            z_pass(1)
            ssb1 = s_mm(1, xp1)
            s_squash(1, it, ssb1)

    nc.compile()
    return nc


def _get_compiled():
    global _compiled
    if _compiled is None:
        _compiled = _build()
    return _compiled


def _make_consts():
    import ml_dtypes
    cb = np.zeros((128, CB_COLS), dtype=ml_dtypes.bfloat16)
    c32 = np.zeros((128, C32_COLS), dtype=np.float32)
    for q in range(CG):
        for g in range(2):
            cb[q * CO:(q + 1) * CO,
               MASK0 + g * FB + q * BL:MASK0 + g * FB + (q + 1) * BL] = 1.0
        c32[q * CO:(q + 1) * CO, BD0 + q] = 1.0
        c32[q, SEL0 + q * CO:SEL0 + (q + 1) * CO] = 1.0
    cb[:, ONESNL] = 1.0
    c32[0, ONE1C:ONE1C + CG] = 1.0
    c32[:, EXPB1] = 0.0
    c32[:, EXPB2] = 0.0
    return cb, c32


def _prep_w(route_weights: np.ndarray):
    w = np.ascontiguousarray(route_weights, dtype=np.float32)
    w5 = w.reshape(NCAPS, NB, 128, CI, CO)
    import ml_dtypes
    wk = np.ascontiguousarray(
        w5.transpose(2, 3, 1, 0, 4).reshape(128, NCH * NCAPS * CO)
        .astype(ml_dtypes.bfloat16))
    wt = np.ascontiguousarray(
        w5.reshape(2, CG, NB, 128, CI, CO)
        .transpose(1, 5, 0, 4, 2, 3).reshape(KO, 2 * NCH * 128)
        .astype(np.float16))
    return wk, wt


def _prep_x_shard(xs: np.ndarray):
    xf = np.ascontiguousarray(
        xs.reshape(BL, NB, 128, CI).transpose(2, 3, 1, 0).reshape(128, NCH * BL))
    return xf.astype(np.float16)


def _extract_out(raw: np.ndarray) -> np.ndarray:
    """raw [KO, 2, FB] -> [NCAPS, BL, CO] diagonal blocks."""
    out = np.empty((NCAPS, BL, CO), dtype=np.float32)
    for c in range(NCAPS):
        g, cl = divmod(c, CG)
        out[c] = raw[cl * CO:(cl + 1) * CO, g, cl * BL:(cl + 1) * BL].T
    return out


def kernel(x: np.ndarray, route_weights: np.ndarray) -> np.ndarray:
    from concourse.bass_utils import run_bass_kernel_spmd

    nc = _get_compiled()
    x = np.ascontiguousarray(x, dtype=np.float32)
    wk, wt = _prep_w(route_weights)
    cb, c32 = _make_consts()
    in_maps = []
    for ci in range(NCORES):
        import ml_dtypes
        xh16 = _prep_x_shard(x[ci * BL:(ci + 1) * BL])
        blobA = np.ascontiguousarray(
            np.concatenate([cb, xh16.astype(ml_dtypes.bfloat16)], axis=1))
        in_maps.append({"blobA": blobA, "wkb": wk, "xf": xh16,
                        "wt": wt, "c32": c32})
    tdir = os.environ.get("CAPS_TRACE_DIR") or None
    if tdir:
        os.makedirs(tdir, exist_ok=True)
    res = run_bass_kernel_spmd(
        nc, in_maps, list(range(NCORES)), tmpdir=tdir,
        trace=bool(int(os.environ.get("CAPS_TRACE", "0"))))
    outs = [_extract_out(res.results[ci]["out"]) for ci in range(NCORES)]
    full = np.concatenate(outs, axis=1)          # [10, 256, 16]
    if res.exec_time_ns is not None:
        kernel.last_exec_time_ns = res.exec_time_ns
    return full[:, :, None, None, :].astype(np.float32)


kernel.last_exec_time_ns = None


# revision 25
# speedup vs baseline: 1.0142x; 1.0142x over previous
"""CapsuleLayer dynamic-routing kernel for Trainium2 (Bass/Tile), SPMD over 8 cores.

Problem (per full input):
  x:  [256, 1152, 8]   route_weights: [10, 1152, 8, 16]
  priors[c,b,n,o] = sum_i x[b,n,i] * W[c,n,i,o]
  3 routing iterations; logits along o are constant =>
  probs are per-(c,b,n) scalars.  out: [10, 256, 1, 1, 16]

Math (per core, b = 32 local batch):
  logits_t[c,b,n] = priors[c,b,n,:] . a_t[c,b,:], a_t = sum of previous
  squashed outputs.  l = sum_i x * V,  V = sum_o W a  (PE matmul, k=(c,o)).
  s_raw[c,b,o] = sum_{n,i} e^l x W  (PE matmul over k=(i,n); 1/Z folded
  into the squash scale).  exp shift keeps e^l in fp16 range.

v3 engine plan (v1 was Vector-bound at 66% with 1x-rate strided ops):
  - x stored once; every consumer reads it through stride-0 broadcast APs
    (verified supported on both DVE and PE operands) -> no 5x replication
    in HBM or SBUF.
  - V evacuated PSUM->fp16 SBUF on the Scalar engine; Vector runs one
    contiguous fp16 multiply at 2x per group-pass and a 3-level tree-add
    for sum_i (tensor_reduce is 1x-only and was strided).
  - 3 input DMAs (fp16 blob = consts|wk|x, wt, tiny fp32 consts); wt
    streams during iter-0 compute.
  - squash per group so the next group's V-matmuls overlap it on PE.
  - PSUM: vb 3 banks x2, spsum 1 bank, one rotating 1-bank "sm" tile for
    small matmul outputs (8 banks total); PSUM reads all at partition 0.
  - capsules in two groups of 5, packed (c,o) x (c',b); off-diag blocks
    masked in the a accumulator (makes the packed V-matmul exact).
"""

import os
from contextlib import ExitStack

import numpy as np

B, N, CI, CO, NCAPS = 256, 1152, 8, 16, 10
NCORES = 8
BL = B // NCORES          # 32 batch per core
NB = N // 128             # 9 n-blocks
NCH = CI * NB             # 72 k-chunks, j = i*9 + nb
CG = 5                    # capsules per group
KO = CG * CO              # 80 partition rows in (c,o) layout
FB = CG * BL              # 160 free cols in (c',b) layout
NUM_ITERATIONS = 3

# fp16 const-blob column map
CB_COLS = 448
MASK0 = 0                 # mask2 [80, 2*160] at cols 0:320
ONESNL = 320              # ones_nl fp16 [128, 1]
# fp32 consts column map
EXPB1, EXPB2 = 0, 1       # exp shifts (0.0 / -9.5)
BD0 = 2                   # bdones [80, 5]  cols 2:7
SEL0 = 7                  # selc  [5, 80]   cols 7:87
ONE1C = 87                # ones_1c [1, 5]  cols 87:92
C32_COLS = 96

BLOB_COLS = CB_COLS + NCH * NCAPS * CO + NCH * BL   # 448+11520+2304

_compiled = None


def _build():
    import concourse.bacc as bacc
    import concourse.mybir as mybir
    import concourse.tile as tile

    fp32 = mybir.dt.float32
    fp16 = mybir.dt.float16
    bf16 = mybir.dt.bfloat16
    Alu = mybir.AluOpType
    Act = mybir.ActivationFunctionType

    nc = bacc.Bacc("TRN2", target_bir_lowering=False, debug=False)
    blobA_d = nc.dram_tensor("blobA", [128, CB_COLS + NCH * BL], bf16,
                             kind="ExternalInput")
    wkb_d = nc.dram_tensor("wkb", [128, NCH * NCAPS * CO], bf16,
                           kind="ExternalInput")
    xf_d = nc.dram_tensor("xf", [128, NCH * BL], fp16, kind="ExternalInput")
    wt_d = nc.dram_tensor("wt", [KO, 2 * NCH * 128], fp16, kind="ExternalInput")
    c32_d = nc.dram_tensor("c32", [128, C32_COLS], fp32, kind="ExternalInput")
    o_d = nc.dram_tensor("out", [KO, 2, FB], fp32, kind="ExternalOutput")

    with tile.TileContext(nc) as tc, ExitStack() as ctx:
        P = ctx.enter_context(tc.tile_pool(name="persist", bufs=1))
        vbp = ctx.enter_context(tc.tile_pool(name="vbp", bufs=3))
        xvp = ctx.enter_context(tc.tile_pool(name="xvp", bufs=1))
        xpp = ctx.enter_context(tc.tile_pool(name="xpp", bufs=2))
        smallp = ctx.enter_context(tc.tile_pool(name="small", bufs=1))
        psv = ctx.enter_context(tc.tile_pool(name="psv", bufs=2, space="PSUM"))
        pss = ctx.enter_context(tc.tile_pool(name="pss", bufs=1, space="PSUM"))

        # ---- persistent SBUF ----
        blobA = P.tile([128, CB_COLS + NCH * BL], bf16)
        wkt = P.tile([128, NCH, NCAPS * CO], bf16)
        xf = P.tile([128, NCH, BL], fp16)
        c32 = P.tile([128, C32_COLS], fp32)
        wt = P.tile([KO, 2, NCH, 128], fp16)
        xc = P.tile([128, NCH, CG, BL], fp16)  # x replicated over c'
        a_all = P.tile([KO, 2, FB], fp32)
        a_bf = P.tile([KO, 2, FB], fp16)
        elt = P.tile([128, 2, NB, FB], bf16)       # e^l per group

        wk = wkt[:, :, :]                                  # [128, 72, 160]
        xh = blobA[:, CB_COLS:] \
            .rearrange("p (j b) -> p j b", j=NCH)          # [128, 72, 32]
        mask2 = blobA[0:KO, MASK0:MASK0 + 2 * FB]          # [80, 320] bf16
        ones16 = blobA[:, ONESNL:ONESNL + 1]               # [128, 1] bf16
        expb1 = c32[:, EXPB1:EXPB1 + 1]
        expb2 = c32[:, EXPB2:EXPB2 + 1]
        bdones = c32[0:KO, BD0:BD0 + CG]                   # [80, 5] fp32
        selc = c32[0:CG, SEL0:SEL0 + KO]                   # [5, 80] fp32
        ones1c = c32[0:1, ONE1C:ONE1C + CG]                # [1, 5] fp32

        def xbc(j0, j1):
            """bf16 x chunk [128, j1-j0, 5, 32] broadcast over capsules."""
            return xh[:, j0:j1, :].unsqueeze(2).broadcast_to(
                [128, j1 - j0, CG, BL])

        def xbf(j0, j1):
            """fp16 x chunk broadcast over capsules (V path)."""
            return xf[:, j0:j1, :].unsqueeze(2).broadcast_to(
                [128, j1 - j0, CG, BL])

        # ---- input DMAs (two HWDGE queues) ----
        nc.scalar.dma_start(c32[:], c32_d[:])
        nc.sync.dma_start(blobA[:], blobA_d[:])
        H = NCH // 2
        nc.sync.dma_start(wkt[:, 0:H, :], wkb_d[:, 0:H * NCAPS * CO]
                          .rearrange("p (j c) -> p j c", j=H))
        nc.sync.dma_start(wkt[:, H:, :], wkb_d[:, H * NCAPS * CO:]
                          .rearrange("p (j c) -> p j c", j=H))
        nc.scalar.dma_start(xf.rearrange("p j b -> p (j b)"), xf_d[:])
        nc.scalar.dma_start(wt[:, 0, :, :].rearrange("p j n -> p (j n)"),
                            wt_d[:, 0:NCH * 128])
        nc.scalar.dma_start(wt[:, 1, :, :].rearrange("p j n -> p (j n)"),
                            wt_d[:, NCH * 128:])

        for cr in range(CG):
            nc.vector.tensor_scalar_mul(xc[:, :, cr, :], xf[:, :, :], 1.0)

        rz_cur = [None, None]

        def sm_tile():
            return pss.tile([128, 512], fp32, tag="sm", name="sm")

        def squash(it, g, ssb, s2):
            """snp/frp via PE; writes a/a_bf (or output DMA at last iter)."""
            smt = sm_tile()
            snp = smt[0:CG, 0:FB]
            nc.tensor.matmul(snp, bdones, s2, start=True, stop=True)
            snt = smallp.tile([CG, FB], fp32, tag="snt")
            if it == 0:
                zc = 1.0 / N
                nc.vector.tensor_scalar_mul(snt[:], snp, zc * zc)
            else:
                rzc = rz_cur[g][:, :]
                t1 = smallp.tile([CG, FB], fp32, tag="snt_t1")
                nc.vector.tensor_tensor(t1[:], rzc, snp, Alu.mult)
                nc.vector.tensor_tensor(snt[:], t1[:], rzc, Alu.mult)
            sq = smallp.tile([CG, FB], fp32, tag="sq")
            nc.scalar.sqrt(sq[:], snt[:])
            den = smallp.tile([CG, FB], fp32, tag="den")
            nc.vector.tensor_scalar_add(den[:], snt[:], 1.0)
            rden = smallp.tile([CG, FB], fp32, tag="rden")
            nc.vector.reciprocal(rden[:], den[:])
            g_t = smallp.tile([CG, FB], fp32, tag="g")
            nc.vector.tensor_tensor(g_t[:], sq[:], rden[:], Alu.mult)
            f = smallp.tile([CG, FB], fp32, tag="f")
            if it == 0:
                nc.vector.tensor_scalar_mul(f[:], g_t[:], 1.0 / N)
            else:
                nc.vector.tensor_tensor(f[:], g_t[:], rz_cur[g][:, :],
                                        Alu.mult)
            smf = sm_tile()
            frp = smf[0:KO, 0:FB]
            nc.tensor.matmul(frp, selc, f[:], start=True, stop=True)
            ov = smallp.tile([KO, FB], fp32, tag="ov")
            nc.vector.tensor_tensor(ov[:], ssb, frp, Alu.mult)
            aq = a_all[:, g, :]
            if it == 0:
                nc.vector.tensor_tensor(aq, ov[:],
                                        mask2[:, g * FB:(g + 1) * FB], Alu.mult)
                nc.scalar.copy(a_bf[:, g, :], aq)
            elif it < NUM_ITERATIONS - 1:
                t2 = smallp.tile([KO, FB], fp32, tag="aq_t")
                nc.vector.tensor_tensor(t2[:], aq, ov[:], Alu.add)
                nc.vector.tensor_tensor(aq, t2[:],
                                        mask2[:, g * FB:(g + 1) * FB], Alu.mult)
                nc.scalar.copy(a_bf[:, g, :], aq)
            else:
                nc.sync.dma_start(o_d[:, g, :], ov[:])

        # ---- PE warm-up: dummy matmuls on c32 while input DMAs stream;
        # keeps HAM at K=8/8 so iter-0 runs at 2.4 GHz ----
        smw = sm_tile()
        for r in range(48):
            nc.tensor.matmul(smw[0:80, 0:96], c32[:, 0:80], c32[:, 0:96],
                             start=True, stop=True)

        # ================= iteration 0 =================
        # s0[(c,o),(c',b)] = sum_j W^T x  (x broadcast over c'), both groups
        # into one PSUM bank; squash reads only the diagonal blocks.
        # one accumulation chain open per PSUM bank at a time (interleaved
        # chains corrupt each other), so run the groups sequentially
        sps0 = pss.tile([KO, 2 * FB], fp32, tag="spsum")
        for g in (0, 1):
            for j in range(NCH):
                xbj = xh[:, j, :].unsqueeze(1).broadcast_to([128, CG, BL])
                nc.tensor.matmul(sps0[:, g * FB:(g + 1) * FB],
                                 wk[:, j, g * KO:(g + 1) * KO], xbj,
                                 start=(j == 0), stop=(j == NCH - 1))
            ssb = smallp.tile([KO, FB], fp32, tag="ssb")
            nc.scalar.copy(ssb[:], sps0[:, g * FB:(g + 1) * FB])
            s2 = smallp.tile([KO, FB], fp32, tag="s2")
            nc.vector.tensor_tensor(s2[:], ssb[:], ssb[:], Alu.mult)
            squash(0, g, ssb[:], s2[:])

        # ================= iterations 1, 2 =================
        zsb_cur = [None]
        sps_cur = [None]

        def l_pass(g, it, filler=None):
            xv = xvp.tile([128, CI, NB * FB], fp16, tag="xv")
            for i in range(CI):
                vb = psv.tile([128, 3, 512], fp32, tag="vb")
                for nbg in range(3):
                    for k in range(3):
                        j = i * NB + nbg * 3 + k
                        nc.tensor.matmul(vb[:, nbg, k * FB:(k + 1) * FB],
                                         wt[:, g, j, :], a_bf[:, g, :],
                                         start=True, stop=True)
                vbf = vbp.tile([128, NB * FB], fp16, tag="vbf")
                nc.scalar.copy(
                    vbf.rearrange("p (a b) -> p a b", a=3),
                    vb[:, :, 0:3 * FB])
                # xv_i = x * V_i, contiguous fp16 at 2x
                nc.vector.tensor_tensor(
                    xv[:, i, :].rearrange("p (a f) -> p a f", a=NB),
                    xc[:, i * NB:(i + 1) * NB, :, :]
                    .rearrange("p j c b -> p j (c b)"),
                    vbf.rearrange("p (a f) -> p a f", a=NB),
                    Alu.mult)
                if filler is not None:
                    filler(i)   # independent DVE work fills the evac wait
            # tree-add over i: [8, 9*160]
            xvi = xv.rearrange("p i f -> p (i f)") \
                    .rearrange("p (i f) -> p i f", i=CI)
            t1 = xvp.tile([128, 4, NB * FB], fp16, tag="t1")
            nc.vector.tensor_tensor(t1[:], xvi[:, 0:4, :], xvi[:, 4:8, :],
                                    Alu.add)
            nc.vector.tensor_tensor(xvi[:, 0:2, :], t1[:, 0:2, :],
                                    t1[:, 2:4, :], Alu.add)
            nc.vector.tensor_tensor(xvi[:, 2, :], xvi[:, 0, :], xvi[:, 1, :],
                                    Alu.add)
            nc.scalar.activation(
                elt[:, g, :, :].rearrange("p a b -> p (a b)"), xvi[:, 2, :],
                Act.Exp, bias=(expb1 if it == 1 else expb2))

        def z_pass(g):
            # Z[g] = sum_n e^l, then rz[g] = 1/Z broadcast to CG rows;
            # per group so each squash chain can start early
            smz = sm_tile()
            zpg = smz[0:1, 0:FB]
            for nb in range(NB):
                nc.tensor.matmul(zpg, ones16, elt[:, g, nb, :],
                                 start=(nb == 0), stop=(nb == NB - 1))
            zsb = smallp.tile([1, FB], fp32, tag="zsb" + str(g), name="zsb")
            nc.scalar.copy(zsb[:], zpg)
            rzg = smallp.tile([1, FB], fp32, tag="rzg" + str(g), name="rzg")
            nc.vector.reciprocal(rzg[:], zsb[:])
            smr = sm_tile()
            rzcp = smr[0:CG, 0:FB]
            nc.tensor.matmul(rzcp, ones1c, rzg[:], start=True, stop=True)
            rzc = smallp.tile([CG, FB], fp32, tag="rzc" + str(g), name="rzc")
            nc.scalar.copy(rzc[:], rzcp)
            rz_cur[g] = rzc

        def xe_op(g, xp, i):
            # xe_i = x * e^l, contiguous (fp16 x bf16 runs at 2x)
            nc.vector.tensor_tensor(
                xp[:, i * NB:(i + 1) * NB, :],
                xc[:, i * NB:(i + 1) * NB, :, :]
                .rearrange("p j c b -> p j (c b)"),
                elt[:, g, :, :], Alu.mult)

        def xe_pass(g):
            xp = xpp.tile([128, NCH, FB], bf16, tag="xp")
            for i in range(CI):
                xe_op(g, xp, i)
            return xp

        def s_mm(g, xp):
            for j in range(NCH):
                nc.tensor.matmul(
                    sps_cur[0][:, g * FB:(g + 1) * FB],
                    wk[:, j, g * KO:(g + 1) * KO], xp[:, j, :],
                    start=(j == 0), stop=(j == NCH - 1))
            ssb = smallp.tile([KO, FB], fp32, tag="ssb" + str(g))
            nc.scalar.copy(ssb[:], sps_cur[0][:, g * FB:(g + 1) * FB])
            return ssb

        def s_squash(g, it, ssb):
            s2 = smallp.tile([KO, FB], fp32, tag="s2")
            nc.vector.tensor_tensor(s2[:], ssb[:], ssb[:], Alu.mult)
            squash(it, g, ssb[:], s2[:])

        for it in (1, 2):
            sps_cur[0] = pss.tile([KO, 2 * FB], fp32, tag="spsum",
                                  name="sps")
            # V0,V1 dense on PE; xe0 fills l1's evac-paced DVE gaps;
            # per-group rz lets squash0 run early so a_bf0 is ready
            # before the next iteration's V0 matmuls
            l_pass(0, it)
            xp0 = xpp.tile([128, NCH, FB], bf16, tag="xp", name="xp0")
            l_pass(1, it, filler=lambda i: xe_op(0, xp0, i))
            z_pass(0)
            ssb0 = s_mm(0, xp0)
            s_squash(0, it, ssb0)
            xp1 = xe_pass(1)
            z_pass(1)
            ssb1 = s_mm(1, xp1)
            s_squash(1, it, ssb1)

    nc.compile()
    return nc


def _get_compiled():
    global _compiled
    if _compiled is None:
        _compiled = _build()
    return _compiled


def _make_consts():
    import ml_dtypes
    cb = np.zeros((128, CB_COLS), dtype=ml_dtypes.bfloat16)
    c32 = np.zeros((128, C32_COLS), dtype=np.float32)
    for q in range(CG):
        for g in range(2):
            cb[q * CO:(q + 1) * CO,
               MASK0 + g * FB + q * BL:MASK0 + g * FB + (q + 1) * BL] = 1.0
        c32[q * CO:(q + 1) * CO, BD0 + q] = 1.0
        c32[q, SEL0 + q * CO:SEL0 + (q + 1) * CO] = 1.0
    cb[:, ONESNL] = 1.0
    c32[0, ONE1C:ONE1C + CG] = 1.0
    c32[:, EXPB1] = 0.0
    c32[:, EXPB2] = 0.0
    return cb, c32


def _prep_w(route_weights: np.ndarray):
    w = np.ascontiguousarray(route_weights, dtype=np.float32)
    w5 = w.reshape(NCAPS, NB, 128, CI, CO)
    import ml_dtypes
    wk = np.ascontiguousarray(
        w5.transpose(2, 3, 1, 0, 4).reshape(128, NCH * NCAPS * CO)
        .astype(ml_dtypes.bfloat16))
    wt = np.ascontiguousarray(
        w5.reshape(2, CG, NB, 128, CI, CO)
        .transpose(1, 5, 0, 4, 2, 3).reshape(KO, 2 * NCH * 128)
        .astype(np.float16))
    return wk, wt


def _prep_x_shard(xs: np.ndarray):
    xf = np.ascontiguousarray(
        xs.reshape(BL, NB, 128, CI).transpose(2, 3, 1, 0).reshape(128, NCH * BL))
    return xf.astype(np.float16)


def _extract_out(raw: np.ndarray) -> np.ndarray:
    """raw [KO, 2, FB] -> [NCAPS, BL, CO] diagonal blocks."""
    out = np.empty((NCAPS, BL, CO), dtype=np.float32)
    for c in range(NCAPS):
        g, cl = divmod(c, CG)
        out[c] = raw[cl * CO:(cl + 1) * CO, g, cl * BL:(cl + 1) * BL].T
    return out


def kernel(x: np.ndarray, route_weights: np.ndarray) -> np.ndarray:
    from concourse.bass_utils import run_bass_kernel_spmd

    nc = _get_compiled()
    x = np.ascontiguousarray(x, dtype=np.float32)
    wk, wt = _prep_w(route_weights)
    cb, c32 = _make_consts()
    in_maps = []
    for ci in range(NCORES):
        import ml_dtypes
        xh16 = _prep_x_shard(x[ci * BL:(ci + 1) * BL])
        blobA = np.ascontiguousarray(
            np.concatenate([cb, xh16.astype(ml_dtypes.bfloat16)], axis=1))
        in_maps.append({"blobA": blobA, "wkb": wk, "xf": xh16,
                        "wt": wt, "c32": c32})
    tdir = os.environ.get("CAPS_TRACE_DIR") or None
    if tdir:
        os.makedirs(tdir, exist_ok=True)
    res = run_bass_kernel_spmd(
        nc, in_maps, list(range(NCORES)), tmpdir=tdir,
        trace=bool(int(os.environ.get("CAPS_TRACE", "0"))))
    outs = [_extract_out(res.results[ci]["out"]) for ci in range(NCORES)]
    full = np.concatenate(outs, axis=1)          # [10, 256, 16]
    if res.exec_time_ns is not None:
        kernel.last_exec_time_ns = res.exec_time_ns
    return full[:, :, None, None, :].astype(np.float32)


kernel.last_exec_time_ns = None
